# revision 29
# baseline (speedup 1.0000x reference)
"""Tensor-parallel dense transformer (4-layer, D=1024, H=16, F=4096, S=2048,
V=32000 tied lm_head) on 8 Trainium2 NeuronCores via Bass/Tile.

v2: d-major residual stream (hiddenT, fp16) with transposed RMSNorm (no DMA
transposes), residual folded into the AllReduce inputs via fused
scalar_tensor_tensor evacuation, kc-pair-batched softmax exp, Silu-fused FFN,
reciprocal_approx_fast for softmax denominators, and s-half pipelining so each
AllReduce overlaps trailing compute.

Sharding (Megatron TP over 8 cores):
  - QKV: output dim (heads) sharded -> 2 heads/core (EL=128 cols)
  - o_proj / down_proj: input dim sharded, partial sums (+resid/8) AllReduced
  - gate/up: F sharded -> FL=512 cols/core
  - lm_head: vocab sharded -> VL=4000 logits/core, host concat

kernel(**inputs) takes the FULL unsharded inputs (as reference.setup_inputs)
and returns full logits [B, S, V] fp32.
"""
import sys
sys.path.insert(0, "/opt/trn_rl_repo")

import numpy as np
import ml_dtypes
from contextlib import ExitStack

import concourse.bass as bass
import concourse.mybir as mybir
import concourse.tile as tile
from concourse import bacc
from concourse.bass import ts

BF = np.float16
F32 = mybir.dt.float32
BF16 = mybir.dt.float16
AF = mybir.ActivationFunctionType
ALU = mybir.AluOpType

V, D, H, F, L, S, B = 32000, 1024, 16, 4096, 4, 2048, 1
NC_CORES = 8
DEBUG = False
ROPE_BASE = 10000.0
EPS = 1e-6
MASK_NEG = -30000.0


def _dims():
    HD = 64
    HL = H // NC_CORES          # heads per core
    EL = HL * HD                # local qkv width
    FL = F // NC_CORES          # local ffn width
    VL = V // NC_CORES          # local vocab
    NT = S // 128               # s-tiles
    NSC = S // 512              # 512-col s-chunks
    ND = D // 128               # d-chunks
    NFT = FL // 128             # f-tiles
    return HD, HL, EL, FL, VL, NT, NSC, ND, NFT


NCH = 4                          # s-chunks per AllReduce phase
SW = S // NCH                    # 512


def build_nc():
    HD, HL, EL, FL, VL, NT, NSC, ND, NFT = _dims()
    nc = bacc.Bacc("TRN2", target_bir_lowering=False, debug=False,
                   num_devices=NC_CORES)

    hid_ext = nc.dram_tensor("hid0T", [D, S], BF16, kind="ExternalInput")
    wq_ext = nc.dram_tensor("wqT", [L, D, EL], BF16, kind="ExternalInput")
    wk_ext = nc.dram_tensor("wkT", [L, D, EL], BF16, kind="ExternalInput")
    wqp_ext = nc.dram_tensor("wqpT", [L, D, EL], BF16, kind="ExternalInput")
    wkp_ext = nc.dram_tensor("wkpT", [L, D, EL], BF16, kind="ExternalInput")
    wv_ext = nc.dram_tensor("wvT", [L, D, EL], BF16, kind="ExternalInput")
    wo_ext = nc.dram_tensor("woT", [L, EL, D], BF16, kind="ExternalInput")
    wg_ext = nc.dram_tensor("wgT", [L, D, FL], BF16, kind="ExternalInput")
    wu_ext = nc.dram_tensor("wuT", [L, D, FL], BF16, kind="ExternalInput")
    wd_ext = nc.dram_tensor("wdT", [L, FL, D], BF16, kind="ExternalInput")
    embT_ext = nc.dram_tensor("embT", [D, VL], BF16, kind="ExternalInput")
    cos_ext = nc.dram_tensor("cosT", [EL, S], BF16, kind="ExternalInput")
    sin_ext = nc.dram_tensor("sinT", [EL, S], BF16, kind="ExternalInput")
    mask_ext = nc.dram_tensor("maskT", [4, 128, 512], BF16, kind="ExternalInput")
    logits_ext = nc.dram_tensor("logits", [S, VL], BF16, kind="ExternalOutput")

    cc_a_in = nc.dram_tensor("cc_a_in", [NCH, D, SW], BF16)
    cc_a_out = nc.dram_tensor("cc_a_out", [NCH, D, SW], BF16, addr_space="Shared")
    cc_f_in = nc.dram_tensor("cc_f_in", [NCH, D, SW], BF16)
    cc_f_out = nc.dram_tensor("cc_f_out", [NCH, D, SW], BF16, addr_space="Shared")
    RG = [list(range(NC_CORES))]

    SCH = SW // 512              # 512-chunks per half (2)
    TH = SW // 128               # 128-tiles per half (8)

    dbg_tensors = {}

    with tile.TileContext(nc) as tc, ExitStack() as ctx:

        def dbg(name, ap):
            if not DEBUG or name in dbg_tensors:
                return
            ext = nc.dram_tensor("dbg_" + name, list(ap.shape), ap.dtype,
                                 kind="ExternalOutput")
            nc.sync.dma_start(ext[...], ap)
            dbg_tensors[name] = ext
        const_p = ctx.enter_context(tc.tile_pool(name="const", bufs=1))
        persist_p = ctx.enter_context(tc.tile_pool(name="persist", bufs=1))
        work_p = ctx.enter_context(tc.tile_pool(name="work", bufs=2))

        hT = persist_p.tile([128, ND, S], BF16)     # residual stream, d-major
        nc.sync.dma_start(hT[:], hid_ext[:, :].rearrange("(c p) s -> p c s", p=128))
        xT = persist_p.tile([128, ND, S], BF16)     # normed input, d-major

        cos_sb = const_p.tile([EL, S], BF16)
        nc.sync.dma_start(cos_sb[:], cos_ext[:, :])
        sin_sb = const_p.tile([EL, S], BF16)
        nc.sync.dma_start(sin_sb[:], sin_ext[:, :])
        mask_sb = const_p.tile([128, 4, 512], BF16)
        nc.sync.dma_start(mask_sb[:], mask_ext[:, :, :].rearrange("i p b -> p i b"))
        ones_sb = const_p.tile([128, 1], BF16)
        nc.gpsimd.memset(ones_sb[:], 1.0)
        onesr = const_p.tile([1, 128], F32)
        nc.gpsimd.memset(onesr[:], 1.0)
        eps1 = const_p.tile([1, 1], F32)
        nc.gpsimd.memset(eps1[:], EPS)

        VH = VL // 2
        emb_p = ctx.enter_context(tc.tile_pool(name="embp", bufs=1))
        emb_tiles = {}

        def norm_half(g):
            """xT[:, :, g-half] = hT / rms(hT) for the s-columns of half g."""
            gsl = ts(g, SW)
            with tc.tile_pool(name=f"nps", bufs=1, space="PSUM") as nps:
                ssq = nps.tile([1, SW], F32, tag="ssq")
                sqs = []
                for dc in range(ND):
                    sq = work_p.tile([128, SW], BF16, tag="sq", bufs=3)
                    nc.scalar.activation(sq[:], hT[:, dc, gsl], AF.Square)
                    sqs.append(sq)
                for blk in range(SCH):
                    for dc in range(ND):
                        nc.tensor.matmul(ssq[0:1, ts(blk, 512)], ones_sb[:],
                                         sqs[dc][:, ts(blk, 512)],
                                         start=(dc == 0), stop=(dc == ND - 1))
                rms = work_p.tile([1, SW], F32, tag="rms", bufs=1)
                nc.scalar.activation(rms[:], ssq[:], AF.Sqrt, scale=1.0 / D,
                                     bias=eps1[:])
                inv = work_p.tile([1, SW], F32, tag="inv", bufs=1)
                nc.vector.reciprocal_approx_fast(inv[:], rms[:])
                binv = work_p.tile([128, SW], F32, tag="binv_sb", bufs=2)
                nc.gpsimd.partition_broadcast(binv[:], inv[:], channels=128)
                for dc in range(ND):
                    nc.vector.tensor_tensor(xT[:, dc, gsl], hT[:, dc, gsl],
                                            binv[:], ALU.mult)

        # initial norm (layer-0 attn input; attn_norm_w folded into Wq/Wk/Wv)
        for g in range(NCH):
            norm_half(g)
        dbg("xT0", xT[:])

        with ExitStack() as lctx:
            loop_p = lctx.enter_context(tc.tile_pool(name="loop", bufs=1))
            w_p = lctx.enter_context(tc.tile_pool(name="wts", bufs=1))

            qsb = loop_p.tile([EL, S], BF16)
            ksb = loop_p.tile([EL, S], BF16)
            o_in = loop_p.tile([EL, S], BF16)
            v_store = loop_p.tile([128, NT, HL, 65], BF16)
            nc.gpsimd.memset(v_store[:, :, :, 64:65], 1.0)

            def rope(src_ps, perm_ps, dst, g):
                # src_ps/perm_ps: [128, SW] fp32 psum (raw and 32-block-swapped
                # projections, both computed on PE); dst cols of half g
                for scc in range(SCH):
                    sl = ts(g * SCH + scc, 512)     # S-space slice
                    pl = ts(scc, 512)               # psum slice
                    tq = work_p.tile([128, 512], BF16, tag="ropet")
                    nc.vector.tensor_tensor(tq[:], src_ps[:, pl], cos_sb[:, sl],
                                            ALU.mult)
                    u = work_p.tile([128, 512], BF16, tag="ropeu")
                    nc.vector.tensor_tensor(u[:], perm_ps[:, pl], sin_sb[:, sl],
                                            ALU.mult)
                    nc.vector.tensor_tensor(dst[:, sl], tq[:], u[:], ALU.add)

            def oproj_and_ar(wo_sb, pool, g):
                par = work_p.tile([128, ND, SW], BF16, tag="par", bufs=1)
                for et in range(ND):
                    for scc in range(SCH):
                        sc = g * SCH + scc
                        ppt = pool.tile([128, 512], F32, tag="pps")
                        nc.tensor.matmul(ppt[:], wo_sb[:, ts(et, 128)],
                                         o_in[:, ts(sc, 512)], start=True, stop=True)
                        nc.vector.scalar_tensor_tensor(
                            par[:, et, ts(scc, 512)], hT[:, et, ts(sc, 512)],
                            1.0 / NC_CORES, ppt[:], ALU.mult, ALU.add)
                nc.sync.dma_start(cc_a_in[g].rearrange("(c p) s -> p c s", p=128),
                                  par[:])
                nc.gpsimd.collective_compute(
                    "AllReduce", ALU.add, replica_groups=RG,
                    ins=[cc_a_in[g].opt()], outs=[cc_a_out[g].opt()])

            def qkv_chunk(qkv_ps, g, W, qk_bufs=4):
                qps = qkv_ps.tile([128, SW], F32, tag="qk", bufs=qk_bufs,
                                  name="qps")
                qpps = qkv_ps.tile([128, SW], F32, tag="qk", bufs=qk_bufs,
                                   name="qpps")
                for dc in range(ND):
                    for scc in range(SCH):
                        nc.tensor.matmul(qps[:, ts(scc, 512)], W["wq"][:, dc, :],
                                         xT[:, dc, ts(g * SCH + scc, 512)],
                                         start=(dc == 0), stop=(dc == ND - 1))
                        nc.tensor.matmul(qpps[:, ts(scc, 512)], W["wqp"][:, dc, :],
                                         xT[:, dc, ts(g * SCH + scc, 512)],
                                         start=(dc == 0), stop=(dc == ND - 1))
                rope(qps, qpps, qsb, g)
                kps = qkv_ps.tile([128, SW], F32, tag="qk", bufs=qk_bufs,
                                  name="kps")
                kpps = qkv_ps.tile([128, SW], F32, tag="qk", bufs=qk_bufs,
                                   name="kpps")
                for dc in range(ND):
                    for scc in range(SCH):
                        nc.tensor.matmul(kps[:, ts(scc, 512)], W["wk"][:, dc, :],
                                         xT[:, dc, ts(g * SCH + scc, 512)],
                                         start=(dc == 0), stop=(dc == ND - 1))
                        nc.tensor.matmul(kpps[:, ts(scc, 512)], W["wkp"][:, dc, :],
                                         xT[:, dc, ts(g * SCH + scc, 512)],
                                         start=(dc == 0), stop=(dc == ND - 1))
                rope(kps, kpps, ksb, g)
                vps = qkv_ps.tile([128, TH, HL, 64], F32, tag="vv", bufs=2,
                                  name="vps")
                for tt in range(TH):
                    t = g * TH + tt
                    for dc in range(ND):
                        nc.tensor.matmul(vps[:, tt, :, :],
                                         xT[:, dc, ts(t, 128)], W["wv"][:, dc, :],
                                         start=(dc == 0), stop=(dc == ND - 1))
                for tt in range(TH):
                    t = g * TH + tt
                    nc.vector.tensor_copy(v_store[:, t, :, 0:64],
                                          vps[:, tt, :, :])

            def attn_j(j, sc_ps, av_ps, op_ps, wo_sb):
                np_pairs = 2 * j + 2
                for h in range(HL):
                    hb = 64 * h
                    avp = av_ps.tile([65, 512], F32, tag="av", name="avp")
                    for p in range(np_pairs):
                        kc0, kc1 = 2 * p, 2 * p + 1
                        scp = sc_ps.tile([128, 2, 512], F32, tag="sc",
                                         name="scp")
                        nc.tensor.matmul(scp[:, 0, :],
                                         ksb[hb:hb + 64, ts(kc0, 128)],
                                         qsb[hb:hb + 64, ts(j, 512)],
                                         start=True, stop=True)
                        nc.tensor.matmul(scp[:, 1, :],
                                         ksb[hb:hb + 64, ts(kc1, 128)],
                                         qsb[hb:hb + 64, ts(j, 512)],
                                         start=True, stop=True)
                        psb = work_p.tile([128, 2, 512], BF16, tag="p", bufs=3)
                        nc.scalar.activation(psb[:], scp[:], AF.Exp,
                                             scale=0.125)
                        if p >= 2 * j:      # diagonal pairs: causal mask
                            i0 = 2 * (p - 2 * j)
                            nc.vector.tensor_tensor(
                                psb[:], psb[:],
                                mask_sb[:, i0:i0 + 2, :], ALU.mult)
                        nc.tensor.matmul(avp[:], v_store[:, kc0, h, :],
                                         psb[:, 0, :],
                                         start=(p == 0), stop=False)
                        nc.tensor.matmul(avp[:], v_store[:, kc1, h, :],
                                         psb[:, 1, :],
                                         start=False, stop=(p == np_pairs - 1))
                    srow = work_p.tile([1, 512], F32, tag="srow", bufs=1)
                    nc.vector.tensor_copy(srow[:], avp[64:65, :])
                    srec = work_p.tile([1, 512], F32, tag="srec", bufs=1)
                    nc.vector.reciprocal_approx_fast(srec[:], srow[:])
                    bcsb = work_p.tile([64, 512], F32, tag="bcsb", bufs=1)
                    nc.gpsimd.partition_broadcast(bcsb[:], srec[:], channels=64)
                    nc.vector.tensor_tensor(o_in[hb:hb + 64, ts(j, 512)],
                                            avp[0:64, :], bcsb[:], ALU.mult)
                oproj_and_ar(wo_sb, op_ps, j)

            def load_qkv_weights(l):
                wq_sb = w_p.tile([128, ND, EL], BF16, tag="wq", name="wq_sb")
                nc.sync.dma_start(wq_sb[:], wq_ext[l].rearrange("(c p) e -> p c e", p=128))
                wk_sb = w_p.tile([128, ND, EL], BF16, tag="wk", name="wk_sb")
                nc.sync.dma_start(wk_sb[:], wk_ext[l].rearrange("(c p) e -> p c e", p=128))
                wqp_sb = w_p.tile([128, ND, EL], BF16, tag="wqp", name="wqp_sb")
                nc.sync.dma_start(wqp_sb[:], wqp_ext[l].rearrange("(c p) e -> p c e", p=128))
                wkp_sb = w_p.tile([128, ND, EL], BF16, tag="wkp", name="wkp_sb")
                nc.sync.dma_start(wkp_sb[:], wkp_ext[l].rearrange("(c p) e -> p c e", p=128))
                wv_sb = w_p.tile([128, ND, EL], BF16, tag="wv", name="wv_sb")
                nc.sync.dma_start(wv_sb[:], wv_ext[l].rearrange("(c p) e -> p c e", p=128))
                wo_sb = w_p.tile([EL, D], BF16, tag="wo", name="wo_sb")
                nc.sync.dma_start(wo_sb[:], wo_ext[l])
                return {"wq": wq_sb, "wk": wk_sb, "wqp": wqp_sb,
                        "wkp": wkp_sb, "wv": wv_sb, "wo": wo_sb}

            def load_ffn_weights(l, W):
                wg_sb = w_p.tile([128, ND, FL], BF16, tag="wg", name="wg_sb")
                nc.sync.dma_start(wg_sb[:], wg_ext[l].rearrange("(c p) f -> p c f", p=128))
                wu_sb = w_p.tile([128, ND, FL], BF16, tag="wu", name="wu_sb")
                nc.sync.dma_start(wu_sb[:], wu_ext[l].rearrange("(c p) f -> p c f", p=128))
                wd_sb = w_p.tile([128, NFT, D], BF16, tag="wd", name="wd_sb")
                nc.sync.dma_start(wd_sb[:], wd_ext[l].rearrange("(c p) e -> p c e", p=128))
                W.update({"wg": wg_sb, "wu": wu_sb, "wd": wd_sb})

            W = None
            Wn = None
            for l in range(L):
                if l == 1:
                    emb0_sb = emb_p.tile([128, ND, VH], BF16, tag="emb")
                    nc.sync.dma_start(
                        emb0_sb[:],
                        embT_ext[:, 0:VH].rearrange("(c p) v -> p c v", p=128))
                    emb_tiles[0] = emb0_sb
                if l == 0:
                    W = load_qkv_weights(0)
                    load_ffn_weights(0, W)
                else:
                    W = Wn   # qkv/o weights prefetched during layer l-1

                # ---- per chunk: norm (from AR_f of l-1) + QKV ----
                # (for l > 0, chunk 0 was hoisted into layer l-1's FFN phase)
                with tc.tile_pool(name="qkvps", bufs=2, space="PSUM") as qkv_ps:
                    for g in (range(NCH) if l == 0 else range(1, NCH)):
                        if l > 0:
                            nc.gpsimd.dma_start(
                                hT[:, :, ts(g, SW)],
                                cc_f_out[g].rearrange("(c p) s -> p c s", p=128))
                            norm_half(g)
                        qkv_chunk(qkv_ps, g, W)

                # ---- attention + o_proj (per q-chunk pipelined with AR) ----
                # (for l > 0, j=0 + its AR were hoisted into layer l-1)
                with tc.tile_pool(name="scps", bufs=2, space="PSUM") as sc_ps, \
                     tc.tile_pool(name="avps", bufs=2, space="PSUM") as av_ps, \
                     tc.tile_pool(name="opps", bufs=2, space="PSUM") as op_ps:
                    for j in (range(NSC) if l == 0 else range(1, NSC)):
                        attn_j(j, sc_ps, av_ps, op_ps, W["wo"])

                # prefetch next layer's qkv/o weights (their layer-l readers
                # are all issued by now); ffn weights follow after ffn(3)
                if l < L - 1:
                    Wn = load_qkv_weights(l + 1)

                # ---- post-AR_a: norm2 + FFN per half ----
                def ffn_half(g, Wl):
                    gsl = ts(g, SW)
                    gsc = work_p.tile([128, NFT, SW], BF16, tag="gsc", bufs=1)
                    with tc.tile_pool(name="gups", bufs=2, space="PSUM") as gu_ps:
                        for ft in range(NFT):
                            gps = gu_ps.tile([128, SW], F32, tag="gu")
                            for dc in range(ND):
                                for scc in range(SCH):
                                    nc.tensor.matmul(gps[:, ts(scc, 512)],
                                                     Wl["wg"][:, dc, ts(ft, 128)],
                                                     xT[:, dc, ts(g * SCH + scc, 512)],
                                                     start=(dc == 0), stop=(dc == ND - 1))
                            sg = work_p.tile([128, SW], BF16, tag="sg", bufs=2)
                            nc.scalar.activation(sg[:], gps[:], AF.Silu)
                            ups = gu_ps.tile([128, SW], F32, tag="gu")
                            for dc in range(ND):
                                for scc in range(SCH):
                                    nc.tensor.matmul(ups[:, ts(scc, 512)],
                                                     Wl["wu"][:, dc, ts(ft, 128)],
                                                     xT[:, dc, ts(g * SCH + scc, 512)],
                                                     start=(dc == 0), stop=(dc == ND - 1))
                            nc.vector.tensor_tensor(gsc[:, ft, :], ups[:], sg[:],
                                                    ALU.mult)
                    with tc.tile_pool(name="dwps", bufs=2, space="PSUM") as dw_ps:
                        par = work_p.tile([128, ND, SW], BF16, tag="par", bufs=1)
                        for et in range(ND):
                            for scc in range(SCH):
                                dps = dw_ps.tile([128, 512], F32, tag="dw")
                                for fc in range(NFT):
                                    nc.tensor.matmul(dps[:], Wl["wd"][:, fc, ts(et, 128)],
                                                     gsc[:, fc, ts(scc, 512)],
                                                     start=(fc == 0), stop=(fc == NFT - 1))
                                nc.vector.scalar_tensor_tensor(
                                    par[:, et, ts(scc, 512)],
                                    hT[:, et, ts(g * SCH + scc, 512)],
                                    1.0 / NC_CORES, dps[:], ALU.mult, ALU.add)
                        nc.sync.dma_start(
                            cc_f_in[g].rearrange("(c p) s -> p c s", p=128), par[:])
                    nc.gpsimd.collective_compute(
                        "AllReduce", ALU.add, replica_groups=RG,
                        ins=[cc_f_in[g].opt()], outs=[cc_f_out[g].opt()])

                for g in range(NCH):
                    nc.gpsimd.dma_start(
                        hT[:, :, ts(g, SW)],
                        cc_a_out[g].rearrange("(c p) s -> p c s", p=128))
                    norm_half(g)
                    ffn_half(g, W)
                    if l < L - 1 and g == 2:
                        # hoist next layer chunk-0 norm + QKV behind ffn(3):
                        # AR_f(0) completed during ffn(1)/ffn(2)
                        nc.gpsimd.dma_start(
                            hT[:, :, ts(0, SW)],
                            cc_f_out[0].rearrange("(c p) s -> p c s", p=128))
                        norm_half(0)
                        with tc.tile_pool(name="hqps", bufs=2,
                                          space="PSUM") as hq_ps:
                            qkv_chunk(hq_ps, 0, Wn)
                    if l < L - 1 and g == 3:
                        # hoist next layer attention(0) + its AllReduce so the
                        # collective engine stays busy across the boundary
                        with tc.tile_pool(name="hscps", bufs=2, space="PSUM") as hsc_ps, \
                             tc.tile_pool(name="havps", bufs=2, space="PSUM") as hav_ps, \
                             tc.tile_pool(name="hopps", bufs=2, space="PSUM") as hop_ps:
                            attn_j(0, hsc_ps, hav_ps, hop_ps, Wn["wo"])
                        load_ffn_weights(l + 1, Wn)

        # ---- lm_head (final_norm_w folded into embT); vocab in halves ----
        vchunks = []
        vv = 0
        while vv < VH:
            vchunks.append((vv, min(512, VH - vv)))
            vv += 512
        TPG = NT // NCH

        def lm_t(lps, t, v0, emb_sb):
            lp = lps.tile([128, VH], F32, tag="lm")
            for dc in range(ND):
                for (vv, vn) in vchunks:
                    nc.tensor.matmul(lp[:, vv:vv + vn],
                                     xT[:, dc, ts(t, 128)],
                                     emb_sb[:, dc, vv:vv + vn],
                                     start=(dc == 0), stop=(dc == ND - 1))
            lsb = work_p.tile([128, VH], BF16, tag="lsb", bufs=1)
            nc.scalar.activation(lsb[:, 0:1024], lp[:, 0:1024], AF.Copy)
            nc.vector.tensor_copy(lsb[:, 1024:VH], lp[:, 1024:VH])
            nc.sync.dma_start(logits_ext[ts(t, 128), v0:v0 + VH], lsb[:])

        for g in range(NCH):
            nc.gpsimd.dma_start(
                hT[:, :, ts(g, SW)],
                cc_f_out[g].rearrange("(c p) s -> p c s", p=128))
            norm_half(g)
            with tc.tile_pool(name="lmps", bufs=2, space="PSUM") as lps:
                for t in range(g * TPG, (g + 1) * TPG):
                    lm_t(lps, t, 0, emb_tiles[0])
        emb1_sb = emb_p.tile([128, ND, VH], BF16, tag="emb")
        nc.sync.dma_start(
            emb1_sb[:], embT_ext[:, VH:VL].rearrange("(c p) v -> p c v", p=128))
        with tc.tile_pool(name="lmps", bufs=2, space="PSUM") as lps:
            for t in range(NT):
                lm_t(lps, t, VH, emb1_sb)

    nc.compile()
    return nc


def host_prep(inputs):
    """Full inputs -> per-core in_maps (list of dicts of np arrays)."""
    HD, HL, EL, FL, VL, NT, NSC, ND, NFT = _dims()
    emb = np.ascontiguousarray(np.asarray(inputs["emb"], np.float32))
    ids = np.asarray(inputs["input_ids"]).reshape(-1)
    hid0T = np.ascontiguousarray(emb[ids].T).astype(BF)   # [D, S]

    anw = np.asarray(inputs["attn_norm_w"], np.float32)
    fnw = np.asarray(inputs["ffn_norm_w"], np.float32)
    finw = np.asarray(inputs["final_norm_w"], np.float32)
    Wq = np.asarray(inputs["Wq"], np.float32)
    Wk = np.asarray(inputs["Wk"], np.float32)
    Wv = np.asarray(inputs["Wv"], np.float32)
    Wo = np.asarray(inputs["Wo"], np.float32)
    Wg = np.asarray(inputs["Wg"], np.float32)
    Wu = np.asarray(inputs["Wu"], np.float32)
    Wd = np.asarray(inputs["Wd"], np.float32)

    # rope tables [EL, S]
    inv_freq = 1.0 / (ROPE_BASE ** (np.arange(0, HD, 2, dtype=np.float32) / HD))
    ang = np.arange(S, dtype=np.float32)[:, None] * inv_freq[None, :]   # [S, HD/2]
    ang = np.concatenate([ang, ang], axis=1)                            # [S, HD]
    cosT = np.cos(ang).T.astype(np.float32)                             # [HD, S]
    sinT = np.sin(ang).T.astype(np.float32)
    sinT[:HD // 2] *= -1.0
    cos_full = np.tile(cosT, (HL, 1)).astype(BF)
    sin_full = np.tile(sinT, (HL, 1)).astype(BF)

    # causal masks [4, 128, 512]: multiplicative (1 = keep, 0 = drop)
    a = np.arange(128)[:, None]
    b = np.arange(512)[None, :]
    maskT = np.stack([(a + 128 * i <= b) for i in range(4)]).astype(np.float32)
    maskT = maskT.astype(BF)

    in_maps = []
    for c in range(NC_CORES):
        er = slice(c * EL, (c + 1) * EL)
        fr = slice(c * FL, (c + 1) * FL)
        vr = slice(c * VL, (c + 1) * VL)
        wqT = np.stack([(Wq[l][er, :] * anw[l][None, :]).T for l in range(L)])
        wkT = np.stack([(Wk[l][er, :] * anw[l][None, :]).T for l in range(L)])
        # 32-block-swapped column permutation (rotate-half partner rows)
        perm = np.concatenate([np.arange(32, 64), np.arange(0, 32),
                               np.arange(96, 128), np.arange(64, 96)])
        wqpT = wqT[:, :, perm]
        wkpT = wkT[:, :, perm]
        wvT = np.stack([(Wv[l][er, :] * anw[l][None, :]).T for l in range(L)])
        woT = np.stack([np.ascontiguousarray(Wo[l][:, er].T) for l in range(L)])
        wgT = np.stack([Wg[l][:, fr] * fnw[l][:, None] for l in range(L)])
        wuT = np.stack([Wu[l][:, fr] * fnw[l][:, None] for l in range(L)])
        wdT = np.stack([Wd[l][fr, :] for l in range(L)])
        embT = np.ascontiguousarray((emb[vr, :] * finw[None, :]).T)
        in_maps.append({
            "hid0T": hid0T,
            "wqT": wqT.astype(BF), "wkT": wkT.astype(BF), "wvT": wvT.astype(BF),
            "wqpT": wqpT.astype(BF), "wkpT": wkpT.astype(BF),
            "woT": woT.astype(BF), "wgT": wgT.astype(BF), "wuT": wuT.astype(BF),
            "wdT": wdT.astype(BF), "embT": embT.astype(BF),
            "cosT": cos_full, "sinT": sin_full, "maskT": maskT,
        })
    return in_maps


_RUNNER = None


def make_runner(nc):
    """Wrap a compiled Bacc module into a jitted 8-core callable."""
    import jax
    from jax.sharding import Mesh, PartitionSpec
    from jax.experimental.shard_map import shard_map
    from concourse.bass2jax import (_bass_exec_p, partition_id_tensor,
                                    install_neuronx_cc_hook)
    import jax.numpy as jnp

    install_neuronx_cc_hook()

    partition_name = nc.partition_id_tensor.name if nc.partition_id_tensor else None
    in_names, out_names, out_avals = [], [], []
    for alloc in nc.m.functions[0].allocations:
        if not isinstance(alloc, mybir.MemoryLocationSet):
            continue
        name = alloc.memorylocations[0].name
        if alloc.kind == "ExternalInput":
            if name != partition_name:
                in_names.append(name)
        elif alloc.kind == "ExternalOutput":
            out_names.append(name)
            out_avals.append(jax.core.ShapedArray(
                tuple(alloc.tensor_shape), mybir.dt.np(alloc.dtype)))
    n_params = len(in_names)
    in_names_all = list(in_names) + list(out_names)
    if partition_name is not None:
        in_names_all.append(partition_name)

    def _body(*args):
        operands = list(args)
        if partition_name is not None:
            operands.append(partition_id_tensor())
        outs = _bass_exec_p.bind(
            *operands,
            out_avals=tuple(out_avals),
            in_names=tuple(in_names_all),
            out_names=tuple(out_names),
            lowering_input_output_aliases=(),
            sim_require_finite=True,
            sim_require_nnan=True,
            nc=nc,
        )
        return tuple(outs)

    devices = jax.devices()[:NC_CORES]
    mesh = Mesh(np.asarray(devices), ("core",))
    n_outs = len(out_names)
    in_specs = (PartitionSpec("core"),) * (n_params + n_outs)
    out_specs = (PartitionSpec("core"),) * len(out_names)
    sharded = jax.jit(shard_map(_body, mesh=mesh, in_specs=in_specs,
                                out_specs=out_specs, check_rep=False),
                      keep_unused=True)

    def zero_outs():
        return [np.zeros((NC_CORES * av.shape[0], *av.shape[1:]), av.dtype)
                for av in out_avals]

    def run(in_maps):
        concat_in = [np.concatenate([np.asarray(in_maps[c][nm])
                                     for c in range(NC_CORES)], axis=0)
                     for nm in in_names]
        out_arrs = sharded(*concat_in, *zero_outs())
        import jax as _jax
        _jax.block_until_ready(out_arrs)
        return [
            {nm: np.asarray(out_arrs[i]).reshape(NC_CORES, *out_avals[i].shape)[c]
             for i, nm in enumerate(out_names)}
            for c in range(NC_CORES)
        ]

    run.zero_outs = zero_outs

    run.sharded = sharded
    run.in_names = in_names
    run.out_names = out_names
    run.out_avals = out_avals
    run.mesh = mesh
    run.nc = nc
    return run


def _get_runner():
    global _RUNNER
    if _RUNNER is None:
        _RUNNER = make_runner(build_nc())
    return _RUNNER


def kernel(**inputs) -> np.ndarray:
    HD, HL, EL, FL, VL, NT, NSC, ND, NFT = _dims()
    in_maps = host_prep(inputs)
    run = _get_runner()
    results = run(in_maps)
    logits = np.concatenate([results[c]["logits"].astype(np.float32)
                             for c in range(NC_CORES)], axis=1)
    return logits.reshape(B, S, V)


# revision 35
# speedup vs baseline: 1.0174x; 1.0174x over previous
"""Tensor-parallel dense transformer (4-layer, D=1024, H=16, F=4096, S=2048,
V=32000 tied lm_head) on 8 Trainium2 NeuronCores via Bass/Tile.

v2: d-major residual stream (hiddenT, fp16) with transposed RMSNorm (no DMA
transposes), residual folded into the AllReduce inputs via fused
scalar_tensor_tensor evacuation, kc-pair-batched softmax exp, Silu-fused FFN,
reciprocal_approx_fast for softmax denominators, and s-half pipelining so each
AllReduce overlaps trailing compute.

Sharding (Megatron TP over 8 cores):
  - QKV: output dim (heads) sharded -> 2 heads/core (EL=128 cols)
  - o_proj / down_proj: input dim sharded, partial sums (+resid/8) AllReduced
  - gate/up: F sharded -> FL=512 cols/core
  - lm_head: vocab sharded -> VL=4000 logits/core, host concat

kernel(**inputs) takes the FULL unsharded inputs (as reference.setup_inputs)
and returns full logits [B, S, V] fp32.
"""
import sys
sys.path.insert(0, "/opt/trn_rl_repo")

import numpy as np
import ml_dtypes
from contextlib import ExitStack

import concourse.bass as bass
import concourse.mybir as mybir
import concourse.tile as tile
from concourse import bacc
from concourse.bass import ts

BF = np.float16
F32 = mybir.dt.float32
BF16 = mybir.dt.float16
AF = mybir.ActivationFunctionType
ALU = mybir.AluOpType

V, D, H, F, L, S, B = 32000, 1024, 16, 4096, 4, 2048, 1
NC_CORES = 8
DEBUG = False
ROPE_BASE = 10000.0
EPS = 1e-6
MASK_NEG = -30000.0


def _dims():
    HD = 64
    HL = H // NC_CORES          # heads per core
    EL = HL * HD                # local qkv width
    FL = F // NC_CORES          # local ffn width
    VL = V // NC_CORES          # local vocab
    NT = S // 128               # s-tiles
    NSC = S // 512              # 512-col s-chunks
    ND = D // 128               # d-chunks
    NFT = FL // 128             # f-tiles
    return HD, HL, EL, FL, VL, NT, NSC, ND, NFT


NCH = 4                          # s-chunks per AllReduce phase
SW = S // NCH                    # 512


def build_nc():
    HD, HL, EL, FL, VL, NT, NSC, ND, NFT = _dims()
    nc = bacc.Bacc("TRN2", target_bir_lowering=False, debug=False,
                   num_devices=NC_CORES)

    hid_ext = nc.dram_tensor("hid0T", [D, S], BF16, kind="ExternalInput")
    wq_ext = nc.dram_tensor("wqT", [L, D, EL], BF16, kind="ExternalInput")
    wk_ext = nc.dram_tensor("wkT", [L, D, EL], BF16, kind="ExternalInput")
    wqp_ext = nc.dram_tensor("wqpT", [L, D, EL], BF16, kind="ExternalInput")
    wkp_ext = nc.dram_tensor("wkpT", [L, D, EL], BF16, kind="ExternalInput")
    wv_ext = nc.dram_tensor("wvT", [L, D, EL], BF16, kind="ExternalInput")
    wo_ext = nc.dram_tensor("woT", [L, EL, D], BF16, kind="ExternalInput")
    wg_ext = nc.dram_tensor("wgT", [L, D, FL], BF16, kind="ExternalInput")
    wu_ext = nc.dram_tensor("wuT", [L, D, FL], BF16, kind="ExternalInput")
    wd_ext = nc.dram_tensor("wdT", [L, FL, D], BF16, kind="ExternalInput")
    embT_ext = nc.dram_tensor("embT", [D, VL], BF16, kind="ExternalInput")
    cos_ext = nc.dram_tensor("cosT", [EL, S], BF16, kind="ExternalInput")
    sin_ext = nc.dram_tensor("sinT", [EL, S], BF16, kind="ExternalInput")
    mask_ext = nc.dram_tensor("maskT", [4, 128, 512], BF16, kind="ExternalInput")
    logits_ext = nc.dram_tensor("logits", [S, VL], BF16, kind="ExternalOutput")

    cc_a_in = nc.dram_tensor("cc_a_in", [NCH, D, SW], BF16)
    cc_a_out = nc.dram_tensor("cc_a_out", [NCH, D, SW], BF16, addr_space="Shared")
    cc_f_in = nc.dram_tensor("cc_f_in", [NCH, D, SW], BF16)
    cc_f_out = nc.dram_tensor("cc_f_out", [NCH, D, SW], BF16, addr_space="Shared")
    RG = [list(range(NC_CORES))]

    SCH = SW // 512              # 512-chunks per half (2)
    TH = SW // 128               # 128-tiles per half (8)

    dbg_tensors = {}

    with tile.TileContext(nc) as tc, ExitStack() as ctx:

        def dbg(name, ap):
            if not DEBUG or name in dbg_tensors:
                return
            ext = nc.dram_tensor("dbg_" + name, list(ap.shape), ap.dtype,
                                 kind="ExternalOutput")
            nc.sync.dma_start(ext[...], ap)
            dbg_tensors[name] = ext
        const_p = ctx.enter_context(tc.tile_pool(name="const", bufs=1))
        persist_p = ctx.enter_context(tc.tile_pool(name="persist", bufs=1))
        work_p = ctx.enter_context(tc.tile_pool(name="work", bufs=2))

        hT = persist_p.tile([128, ND, S], BF16)     # residual stream, d-major
        nc.sync.dma_start(hT[:], hid_ext[:, :].rearrange("(c p) s -> p c s", p=128))
        xT = persist_p.tile([128, ND, S], BF16)     # normed input, d-major

        cos_sb = const_p.tile([EL, S], BF16)
        nc.sync.dma_start(cos_sb[:], cos_ext[:, :])
        sin_sb = const_p.tile([EL, S], BF16)
        nc.sync.dma_start(sin_sb[:], sin_ext[:, :])
        mask_sb = const_p.tile([128, 4, 512], BF16)
        nc.sync.dma_start(mask_sb[:], mask_ext[:, :, :].rearrange("i p b -> p i b"))
        ones_sb = const_p.tile([128, 1], BF16)
        nc.gpsimd.memset(ones_sb[:], 1.0)
        onesr = const_p.tile([1, 128], F32)
        nc.gpsimd.memset(onesr[:], 1.0)
        eps1 = const_p.tile([1, 1], F32)
        nc.gpsimd.memset(eps1[:], EPS)

        VH = VL // 2
        emb_p = ctx.enter_context(tc.tile_pool(name="embp", bufs=1))
        emb_tiles = {}

        def norm_half(g, pool=None):
            """xT[:, :, g-half] = hT / rms(hT) for the s-columns of half g.

            1/rms via Ln+Exp: stays in the natural_log_exp ACT table set
            (shared with attention's Exp) so per-chunk interleaving of norms
            and attention causes no table swaps."""
            gsl = ts(g, SW)
            with ExitStack() as nctx:
                if pool is None:
                    nps = nctx.enter_context(
                        tc.tile_pool(name="nps", bufs=1, space="PSUM"))
                    ssq = nps.tile([1, SW], F32, tag="ssq")
                else:
                    ssq = pool.tile([1, SW], F32, tag="vv", bufs=1, name="ssq")
                sqs = []
                for dc in range(ND):
                    sq = work_p.tile([128, SW], BF16, tag="sq", bufs=3)
                    nc.scalar.activation(sq[:], hT[:, dc, gsl], AF.Square)
                    sqs.append(sq)
                for blk in range(SCH):
                    for dc in range(ND):
                        nc.tensor.matmul(ssq[0:1, ts(blk, 512)], ones_sb[:],
                                         sqs[dc][:, ts(blk, 512)],
                                         start=(dc == 0), stop=(dc == ND - 1))
                lns = work_p.tile([1, SW], F32, tag="lns", bufs=1)
                nc.scalar.activation(lns[:], ssq[:], AF.Ln, scale=1.0 / D,
                                     bias=eps1[:])
                inv = work_p.tile([1, SW], F32, tag="inv", bufs=1)
                nc.scalar.activation(inv[:], lns[:], AF.Exp, scale=-0.5)
                binv = work_p.tile([128, SW], F32, tag="binv_sb", bufs=2)
                nc.gpsimd.partition_broadcast(binv[:], inv[:], channels=128)
                for dc in range(ND):
                    nc.vector.tensor_tensor(xT[:, dc, gsl], hT[:, dc, gsl],
                                            binv[:], ALU.mult)

        # initial norm (layer-0 attn input; attn_norm_w folded into Wq/Wk/Wv)
        for g in range(NCH):
            norm_half(g)
        dbg("xT0", xT[:])

        with ExitStack() as lctx:
            loop_p = lctx.enter_context(tc.tile_pool(name="loop", bufs=1))
            w_p = lctx.enter_context(tc.tile_pool(name="wts", bufs=1))

            qsb = loop_p.tile([EL, S], BF16)
            ksb = loop_p.tile([EL, S], BF16)
            o_in = loop_p.tile([EL, S], BF16)
            v_store = loop_p.tile([128, NT, HL, 65], BF16)
            nc.gpsimd.memset(v_store[:, :, :, 64:65], 1.0)

            def rope(src_ps, perm_ps, dst, g):
                # src_ps/perm_ps: [128, SW] fp32 psum (raw and 32-block-swapped
                # projections, both computed on PE); dst cols of half g
                for scc in range(SCH):
                    sl = ts(g * SCH + scc, 512)     # S-space slice
                    pl = ts(scc, 512)               # psum slice
                    tq = work_p.tile([128, 512], BF16, tag="ropet")
                    nc.vector.tensor_tensor(tq[:], src_ps[:, pl], cos_sb[:, sl],
                                            ALU.mult)
                    u = work_p.tile([128, 512], BF16, tag="ropeu")
                    nc.vector.tensor_tensor(u[:], perm_ps[:, pl], sin_sb[:, sl],
                                            ALU.mult)
                    nc.vector.tensor_tensor(dst[:, sl], tq[:], u[:], ALU.add)

            def oproj_and_ar(wo_sb, pool, g):
                par = work_p.tile([128, ND, SW], BF16, tag="par", bufs=1)
                for et in range(ND):
                    for scc in range(SCH):
                        sc = g * SCH + scc
                        ppt = pool.tile([128, 512], F32, tag="pps")
                        nc.tensor.matmul(ppt[:], wo_sb[:, ts(et, 128)],
                                         o_in[:, ts(sc, 512)], start=True, stop=True)
                        nc.vector.scalar_tensor_tensor(
                            par[:, et, ts(scc, 512)], hT[:, et, ts(sc, 512)],
                            1.0 / NC_CORES, ppt[:], ALU.mult, ALU.add)
                nc.sync.dma_start(cc_a_in[g].rearrange("(c p) s -> p c s", p=128),
                                  par[:])
                nc.gpsimd.collective_compute(
                    "AllReduce", ALU.add, replica_groups=RG,
                    ins=[cc_a_in[g].opt()], outs=[cc_a_out[g].opt()])

            def qkv_chunk(qkv_ps, g, W, qk_bufs=4, vv_bufs=2):
                qps = qkv_ps.tile([128, SW], F32, tag="qk", bufs=qk_bufs,
                                  name="qps")
                qpps = qkv_ps.tile([128, SW], F32, tag="qk", bufs=qk_bufs,
                                   name="qpps")
                for dc in range(ND):
                    for scc in range(SCH):
                        nc.tensor.matmul(qps[:, ts(scc, 512)], W["wq"][:, dc, :],
                                         xT[:, dc, ts(g * SCH + scc, 512)],
                                         start=(dc == 0), stop=(dc == ND - 1))
                        nc.tensor.matmul(qpps[:, ts(scc, 512)], W["wqp"][:, dc, :],
                                         xT[:, dc, ts(g * SCH + scc, 512)],
                                         start=(dc == 0), stop=(dc == ND - 1))
                rope(qps, qpps, qsb, g)
                kps = qkv_ps.tile([128, SW], F32, tag="qk", bufs=qk_bufs,
                                  name="kps")
                kpps = qkv_ps.tile([128, SW], F32, tag="qk", bufs=qk_bufs,
                                   name="kpps")
                for dc in range(ND):
                    for scc in range(SCH):
                        nc.tensor.matmul(kps[:, ts(scc, 512)], W["wk"][:, dc, :],
                                         xT[:, dc, ts(g * SCH + scc, 512)],
                                         start=(dc == 0), stop=(dc == ND - 1))
                        nc.tensor.matmul(kpps[:, ts(scc, 512)], W["wkp"][:, dc, :],
                                         xT[:, dc, ts(g * SCH + scc, 512)],
                                         start=(dc == 0), stop=(dc == ND - 1))
                rope(kps, kpps, ksb, g)
                vps = qkv_ps.tile([128, TH, HL, 64], F32, tag="vv", bufs=vv_bufs,
                                  name="vps")
                for tt in range(TH):
                    t = g * TH + tt
                    for dc in range(ND):
                        nc.tensor.matmul(vps[:, tt, :, :],
                                         xT[:, dc, ts(t, 128)], W["wv"][:, dc, :],
                                         start=(dc == 0), stop=(dc == ND - 1))
                for tt in range(TH):
                    t = g * TH + tt
                    nc.vector.tensor_copy(v_store[:, t, :, 0:64],
                                          vps[:, tt, :, :])

            def attn_j(j, sc_ps, av_ps, op_ps, wo_sb):
                np_pairs = 2 * j + 2
                for h in range(HL):
                    hb = 64 * h
                    avp = av_ps.tile([65, 512], F32, tag="av", bufs=2,
                                     name="avp")
                    for p in range(np_pairs):
                        kc0, kc1 = 2 * p, 2 * p + 1
                        scp = sc_ps.tile([128, 2, 512], F32, tag="sc",
                                         name="scp")
                        nc.tensor.matmul(scp[:, 0, :],
                                         ksb[hb:hb + 64, ts(kc0, 128)],
                                         qsb[hb:hb + 64, ts(j, 512)],
                                         start=True, stop=True)
                        nc.tensor.matmul(scp[:, 1, :],
                                         ksb[hb:hb + 64, ts(kc1, 128)],
                                         qsb[hb:hb + 64, ts(j, 512)],
                                         start=True, stop=True)
                        psb = work_p.tile([128, 2, 512], BF16, tag="p", bufs=3)
                        nc.scalar.activation(psb[:], scp[:], AF.Exp,
                                             scale=0.125)
                        if p >= 2 * j:      # diagonal pairs: causal mask
                            i0 = 2 * (p - 2 * j)
                            nc.vector.tensor_tensor(
                                psb[:], psb[:],
                                mask_sb[:, i0:i0 + 2, :], ALU.mult)
                        nc.tensor.matmul(avp[:], v_store[:, kc0, h, :],
                                         psb[:, 0, :],
                                         start=(p == 0), stop=False)
                        nc.tensor.matmul(avp[:], v_store[:, kc1, h, :],
                                         psb[:, 1, :],
                                         start=False, stop=(p == np_pairs - 1))
                    srow = work_p.tile([1, 512], F32, tag="srow", bufs=1)
                    nc.vector.tensor_copy(srow[:], avp[64:65, :])
                    srec = work_p.tile([1, 512], F32, tag="srec", bufs=1)
                    nc.vector.reciprocal_approx_fast(srec[:], srow[:])
                    bcsb = work_p.tile([64, 512], F32, tag="bcsb", bufs=1)
                    nc.gpsimd.partition_broadcast(bcsb[:], srec[:], channels=64)
                    nc.vector.tensor_tensor(o_in[hb:hb + 64, ts(j, 512)],
                                            avp[0:64, :], bcsb[:], ALU.mult)
                oproj_and_ar(wo_sb, op_ps, j)

            def load_qkv_weights(l):
                wq_sb = w_p.tile([128, ND, EL], BF16, tag="wq", name="wq_sb")
                nc.sync.dma_start(wq_sb[:], wq_ext[l].rearrange("(c p) e -> p c e", p=128))
                wk_sb = w_p.tile([128, ND, EL], BF16, tag="wk", name="wk_sb")
                nc.sync.dma_start(wk_sb[:], wk_ext[l].rearrange("(c p) e -> p c e", p=128))
                wqp_sb = w_p.tile([128, ND, EL], BF16, tag="wqp", name="wqp_sb")
                nc.sync.dma_start(wqp_sb[:], wqp_ext[l].rearrange("(c p) e -> p c e", p=128))
                wkp_sb = w_p.tile([128, ND, EL], BF16, tag="wkp", name="wkp_sb")
                nc.sync.dma_start(wkp_sb[:], wkp_ext[l].rearrange("(c p) e -> p c e", p=128))
                wv_sb = w_p.tile([128, ND, EL], BF16, tag="wv", name="wv_sb")
                nc.sync.dma_start(wv_sb[:], wv_ext[l].rearrange("(c p) e -> p c e", p=128))
                wo_sb = w_p.tile([EL, D], BF16, tag="wo", name="wo_sb")
                nc.sync.dma_start(wo_sb[:], wo_ext[l])
                return {"wq": wq_sb, "wk": wk_sb, "wqp": wqp_sb,
                        "wkp": wkp_sb, "wv": wv_sb, "wo": wo_sb}

            def load_ffn_weights(l, W):
                wg_sb = w_p.tile([128, ND, FL], BF16, tag="wg", name="wg_sb")
                nc.sync.dma_start(wg_sb[:], wg_ext[l].rearrange("(c p) f -> p c f", p=128))
                wu_sb = w_p.tile([128, ND, FL], BF16, tag="wu", name="wu_sb")
                nc.sync.dma_start(wu_sb[:], wu_ext[l].rearrange("(c p) f -> p c f", p=128))
                wd_sb = w_p.tile([128, NFT, D], BF16, tag="wd", name="wd_sb")
                nc.sync.dma_start(wd_sb[:], wd_ext[l].rearrange("(c p) e -> p c e", p=128))
                W.update({"wg": wg_sb, "wu": wu_sb, "wd": wd_sb})

            W = None
            for l in range(L):
                if l == 1:
                    emb0_sb = emb_p.tile([128, ND, VH], BF16, tag="emb")
                    nc.sync.dma_start(
                        emb0_sb[:],
                        embT_ext[:, 0:VH].rearrange("(c p) v -> p c v", p=128))
                    emb_tiles[0] = emb0_sb
                W = load_qkv_weights(l)
                load_ffn_weights(l, W)

                # ---- phase A, interleaved per chunk: norm (from AR_f of
                # l-1) + QKV + attention + o_proj/AR — each chunk's
                # AllReduce fires as soon as that chunk's attention is done,
                # overlapping the next chunk's norm/QKV/attention ----
                with tc.tile_pool(name="aps", bufs=1, space="PSUM") as aps:
                    for g in range(NCH):
                        if l > 0:
                            nc.gpsimd.dma_start(
                                hT[:, :, ts(g, SW)],
                                cc_f_out[g].rearrange("(c p) s -> p c s", p=128))
                            norm_half(g, aps)
                        qkv_chunk(aps, g, W, qk_bufs=2, vv_bufs=1)
                        attn_j(g, aps, aps, aps, W["wo"])

                # ---- post-AR_a: norm2 + FFN per half ----
                def ffn_half(g, Wl):
                    gsl = ts(g, SW)
                    gsc = work_p.tile([128, NFT, SW], BF16, tag="gsc", bufs=1)
                    with tc.tile_pool(name="gups", bufs=2, space="PSUM") as gu_ps:
                        for ft in range(NFT):
                            gps = gu_ps.tile([128, SW], F32, tag="gu")
                            for dc in range(ND):
                                for scc in range(SCH):
                                    nc.tensor.matmul(gps[:, ts(scc, 512)],
                                                     Wl["wg"][:, dc, ts(ft, 128)],
                                                     xT[:, dc, ts(g * SCH + scc, 512)],
                                                     start=(dc == 0), stop=(dc == ND - 1))
                            sg = work_p.tile([128, SW], BF16, tag="sg", bufs=2)
                            nc.scalar.activation(sg[:], gps[:], AF.Silu)
                            ups = gu_ps.tile([128, SW], F32, tag="gu")
                            for dc in range(ND):
                                for scc in range(SCH):
                                    nc.tensor.matmul(ups[:, ts(scc, 512)],
                                                     Wl["wu"][:, dc, ts(ft, 128)],
                                                     xT[:, dc, ts(g * SCH + scc, 512)],
                                                     start=(dc == 0), stop=(dc == ND - 1))
                            nc.vector.tensor_tensor(gsc[:, ft, :], ups[:], sg[:],
                                                    ALU.mult)
                    with tc.tile_pool(name="dwps", bufs=2, space="PSUM") as dw_ps:
                        par = work_p.tile([128, ND, SW], BF16, tag="par", bufs=1)
                        for et in range(ND):
                            for scc in range(SCH):
                                dps = dw_ps.tile([128, 512], F32, tag="dw")
                                for fc in range(NFT):
                                    nc.tensor.matmul(dps[:], Wl["wd"][:, fc, ts(et, 128)],
                                                     gsc[:, fc, ts(scc, 512)],
                                                     start=(fc == 0), stop=(fc == NFT - 1))
                                nc.vector.scalar_tensor_tensor(
                                    par[:, et, ts(scc, 512)],
                                    hT[:, et, ts(g * SCH + scc, 512)],
                                    1.0 / NC_CORES, dps[:], ALU.mult, ALU.add)
                        nc.sync.dma_start(
                            cc_f_in[g].rearrange("(c p) s -> p c s", p=128), par[:])
                    nc.gpsimd.collective_compute(
                        "AllReduce", ALU.add, replica_groups=RG,
                        ins=[cc_f_in[g].opt()], outs=[cc_f_out[g].opt()])

                for g in range(NCH):
                    nc.gpsimd.dma_start(
                        hT[:, :, ts(g, SW)],
                        cc_a_out[g].rearrange("(c p) s -> p c s", p=128))
                    norm_half(g)
                    ffn_half(g, W)

        # ---- lm_head (final_norm_w folded into embT); vocab in halves ----
        vchunks = []
        vv = 0
        while vv < VH:
            vchunks.append((vv, min(512, VH - vv)))
            vv += 512
        TPG = NT // NCH

        def lm_t(lps, t, v0, emb_sb):
            lp = lps.tile([128, VH], F32, tag="lm")
            for dc in range(ND):
                for (vv, vn) in vchunks:
                    nc.tensor.matmul(lp[:, vv:vv + vn],
                                     xT[:, dc, ts(t, 128)],
                                     emb_sb[:, dc, vv:vv + vn],
                                     start=(dc == 0), stop=(dc == ND - 1))
            lsb = work_p.tile([128, VH], BF16, tag="lsb", bufs=1)
            nc.scalar.activation(lsb[:, 0:1024], lp[:, 0:1024], AF.Copy)
            nc.vector.tensor_copy(lsb[:, 1024:VH], lp[:, 1024:VH])
            nc.sync.dma_start(logits_ext[ts(t, 128), v0:v0 + VH], lsb[:])

        for g in range(NCH):
            nc.gpsimd.dma_start(
                hT[:, :, ts(g, SW)],
                cc_f_out[g].rearrange("(c p) s -> p c s", p=128))
            norm_half(g)
            with tc.tile_pool(name="lmps", bufs=2, space="PSUM") as lps:
                for t in range(g * TPG, (g + 1) * TPG):
                    lm_t(lps, t, 0, emb_tiles[0])
        emb1_sb = emb_p.tile([128, ND, VH], BF16, tag="emb")
        nc.sync.dma_start(
            emb1_sb[:], embT_ext[:, VH:VL].rearrange("(c p) v -> p c v", p=128))
        with tc.tile_pool(name="lmps", bufs=2, space="PSUM") as lps:
            for t in range(NT):
                lm_t(lps, t, VH, emb1_sb)

    nc.compile()
    return nc


def host_prep(inputs):
    """Full inputs -> per-core in_maps (list of dicts of np arrays)."""
    HD, HL, EL, FL, VL, NT, NSC, ND, NFT = _dims()
    emb = np.ascontiguousarray(np.asarray(inputs["emb"], np.float32))
    ids = np.asarray(inputs["input_ids"]).reshape(-1)
    hid0T = np.ascontiguousarray(emb[ids].T).astype(BF)   # [D, S]

    anw = np.asarray(inputs["attn_norm_w"], np.float32)
    fnw = np.asarray(inputs["ffn_norm_w"], np.float32)
    finw = np.asarray(inputs["final_norm_w"], np.float32)
    Wq = np.asarray(inputs["Wq"], np.float32)
    Wk = np.asarray(inputs["Wk"], np.float32)
    Wv = np.asarray(inputs["Wv"], np.float32)
    Wo = np.asarray(inputs["Wo"], np.float32)
    Wg = np.asarray(inputs["Wg"], np.float32)
    Wu = np.asarray(inputs["Wu"], np.float32)
    Wd = np.asarray(inputs["Wd"], np.float32)

    # rope tables [EL, S]
    inv_freq = 1.0 / (ROPE_BASE ** (np.arange(0, HD, 2, dtype=np.float32) / HD))
    ang = np.arange(S, dtype=np.float32)[:, None] * inv_freq[None, :]   # [S, HD/2]
    ang = np.concatenate([ang, ang], axis=1)                            # [S, HD]
    cosT = np.cos(ang).T.astype(np.float32)                             # [HD, S]
    sinT = np.sin(ang).T.astype(np.float32)
    sinT[:HD // 2] *= -1.0
    cos_full = np.tile(cosT, (HL, 1)).astype(BF)
    sin_full = np.tile(sinT, (HL, 1)).astype(BF)

    # causal masks [4, 128, 512]: multiplicative (1 = keep, 0 = drop)
    a = np.arange(128)[:, None]
    b = np.arange(512)[None, :]
    maskT = np.stack([(a + 128 * i <= b) for i in range(4)]).astype(np.float32)
    maskT = maskT.astype(BF)

    in_maps = []
    for c in range(NC_CORES):
        er = slice(c * EL, (c + 1) * EL)
        fr = slice(c * FL, (c + 1) * FL)
        vr = slice(c * VL, (c + 1) * VL)
        wqT = np.stack([(Wq[l][er, :] * anw[l][None, :]).T for l in range(L)])
        wkT = np.stack([(Wk[l][er, :] * anw[l][None, :]).T for l in range(L)])
        # 32-block-swapped column permutation (rotate-half partner rows)
        perm = np.concatenate([np.arange(32, 64), np.arange(0, 32),
                               np.arange(96, 128), np.arange(64, 96)])
        wqpT = wqT[:, :, perm]
        wkpT = wkT[:, :, perm]
        wvT = np.stack([(Wv[l][er, :] * anw[l][None, :]).T for l in range(L)])
        woT = np.stack([np.ascontiguousarray(Wo[l][:, er].T) for l in range(L)])
        wgT = np.stack([Wg[l][:, fr] * fnw[l][:, None] for l in range(L)])
        wuT = np.stack([Wu[l][:, fr] * fnw[l][:, None] for l in range(L)])
        wdT = np.stack([Wd[l][fr, :] for l in range(L)])
        embT = np.ascontiguousarray((emb[vr, :] * finw[None, :]).T)
        in_maps.append({
            "hid0T": hid0T,
            "wqT": wqT.astype(BF), "wkT": wkT.astype(BF), "wvT": wvT.astype(BF),
            "wqpT": wqpT.astype(BF), "wkpT": wkpT.astype(BF),
            "woT": woT.astype(BF), "wgT": wgT.astype(BF), "wuT": wuT.astype(BF),
            "wdT": wdT.astype(BF), "embT": embT.astype(BF),
            "cosT": cos_full, "sinT": sin_full, "maskT": maskT,
        })
    return in_maps


_RUNNER = None


def make_runner(nc):
    """Wrap a compiled Bacc module into a jitted 8-core callable."""
    import jax
    from jax.sharding import Mesh, PartitionSpec
    from jax.experimental.shard_map import shard_map
    from concourse.bass2jax import (_bass_exec_p, partition_id_tensor,
                                    install_neuronx_cc_hook)
    import jax.numpy as jnp

    install_neuronx_cc_hook()

    partition_name = nc.partition_id_tensor.name if nc.partition_id_tensor else None
    in_names, out_names, out_avals = [], [], []
    for alloc in nc.m.functions[0].allocations:
        if not isinstance(alloc, mybir.MemoryLocationSet):
            continue
        name = alloc.memorylocations[0].name
        if alloc.kind == "ExternalInput":
            if name != partition_name:
                in_names.append(name)
        elif alloc.kind == "ExternalOutput":
            out_names.append(name)
            out_avals.append(jax.core.ShapedArray(
                tuple(alloc.tensor_shape), mybir.dt.np(alloc.dtype)))
    n_params = len(in_names)
    in_names_all = list(in_names) + list(out_names)
    if partition_name is not None:
        in_names_all.append(partition_name)

    def _body(*args):
        operands = list(args)
        if partition_name is not None:
            operands.append(partition_id_tensor())
        outs = _bass_exec_p.bind(
            *operands,
            out_avals=tuple(out_avals),
            in_names=tuple(in_names_all),
            out_names=tuple(out_names),
            lowering_input_output_aliases=(),
            sim_require_finite=True,
            sim_require_nnan=True,
            nc=nc,
        )
        return tuple(outs)

    devices = jax.devices()[:NC_CORES]
    mesh = Mesh(np.asarray(devices), ("core",))
    n_outs = len(out_names)
    in_specs = (PartitionSpec("core"),) * (n_params + n_outs)
    out_specs = (PartitionSpec("core"),) * len(out_names)
    sharded = jax.jit(shard_map(_body, mesh=mesh, in_specs=in_specs,
                                out_specs=out_specs, check_rep=False),
                      keep_unused=True)

    def zero_outs():
        return [np.zeros((NC_CORES * av.shape[0], *av.shape[1:]), av.dtype)
                for av in out_avals]

    def run(in_maps):
        concat_in = [np.concatenate([np.asarray(in_maps[c][nm])
                                     for c in range(NC_CORES)], axis=0)
                     for nm in in_names]
        out_arrs = sharded(*concat_in, *zero_outs())
        import jax as _jax
        _jax.block_until_ready(out_arrs)
        return [
            {nm: np.asarray(out_arrs[i]).reshape(NC_CORES, *out_avals[i].shape)[c]
             for i, nm in enumerate(out_names)}
            for c in range(NC_CORES)
        ]

    run.zero_outs = zero_outs

    run.sharded = sharded
    run.in_names = in_names
    run.out_names = out_names
    run.out_avals = out_avals
    run.mesh = mesh
    run.nc = nc
    return run


def _get_runner():
    global _RUNNER
    if _RUNNER is None:
        _RUNNER = make_runner(build_nc())
    return _RUNNER


def kernel(**inputs) -> np.ndarray:
    HD, HL, EL, FL, VL, NT, NSC, ND, NFT = _dims()
    in_maps = host_prep(inputs)
    run = _get_runner()
    results = run(in_maps)
    logits = np.concatenate([results[c]["logits"].astype(np.float32)
                             for c in range(NC_CORES)], axis=1)
    return logits.reshape(B, S, V)


# revision 37
# speedup vs baseline: 1.0843x; 1.0658x over previous
"""Tensor-parallel dense transformer (4-layer, D=1024, H=16, F=4096, S=2048,
V=32000 tied lm_head) on 8 Trainium2 NeuronCores via Bass/Tile.

v2: d-major residual stream (hiddenT, fp16) with transposed RMSNorm (no DMA
transposes), residual folded into the AllReduce inputs via fused
scalar_tensor_tensor evacuation, kc-pair-batched softmax exp, Silu-fused FFN,
reciprocal_approx_fast for softmax denominators, and s-half pipelining so each
AllReduce overlaps trailing compute.

Sharding (Megatron TP over 8 cores):
  - QKV: output dim (heads) sharded -> 2 heads/core (EL=128 cols)
  - o_proj / down_proj: input dim sharded, partial sums (+resid/8) AllReduced
  - gate/up: F sharded -> FL=512 cols/core
  - lm_head: vocab sharded -> VL=4000 logits/core, host concat

kernel(**inputs) takes the FULL unsharded inputs (as reference.setup_inputs)
and returns full logits [B, S, V] fp32.
"""
import sys
sys.path.insert(0, "/opt/trn_rl_repo")

import numpy as np
import ml_dtypes
from contextlib import ExitStack

import concourse.bass as bass
import concourse.mybir as mybir
import concourse.tile as tile
from concourse import bacc
from concourse.bass import ts

BF = np.float16
F32 = mybir.dt.float32
BF16 = mybir.dt.float16
AF = mybir.ActivationFunctionType
ALU = mybir.AluOpType

V, D, H, F, L, S, B = 32000, 1024, 16, 4096, 4, 2048, 1
NC_CORES = 8
DEBUG = False
ROPE_BASE = 10000.0
EPS = 1e-6
MASK_NEG = -30000.0


def _dims():
    HD = 64
    HL = H // NC_CORES          # heads per core
    EL = HL * HD                # local qkv width
    FL = F // NC_CORES          # local ffn width
    VL = V // NC_CORES          # local vocab
    NT = S // 128               # s-tiles
    NSC = S // 512              # 512-col s-chunks
    ND = D // 128               # d-chunks
    NFT = FL // 128             # f-tiles
    return HD, HL, EL, FL, VL, NT, NSC, ND, NFT


NCH = 4                          # s-chunks per AllReduce phase
SW = S // NCH                    # 512


def build_nc():
    HD, HL, EL, FL, VL, NT, NSC, ND, NFT = _dims()
    nc = bacc.Bacc("TRN2", target_bir_lowering=False, debug=False,
                   num_devices=NC_CORES)

    hid_ext = nc.dram_tensor("hid0T", [D, S], BF16, kind="ExternalInput")
    wq_ext = nc.dram_tensor("wqT", [L, D, EL], BF16, kind="ExternalInput")
    wk_ext = nc.dram_tensor("wkT", [L, D, EL], BF16, kind="ExternalInput")
    wqp_ext = nc.dram_tensor("wqpT", [L, D, EL], BF16, kind="ExternalInput")
    wkp_ext = nc.dram_tensor("wkpT", [L, D, EL], BF16, kind="ExternalInput")
    wv_ext = nc.dram_tensor("wvT", [L, D, EL], BF16, kind="ExternalInput")
    wo_ext = nc.dram_tensor("woT", [L, EL, D], BF16, kind="ExternalInput")
    wg_ext = nc.dram_tensor("wgT", [L, D, FL], BF16, kind="ExternalInput")
    wu_ext = nc.dram_tensor("wuT", [L, D, FL], BF16, kind="ExternalInput")
    wd_ext = nc.dram_tensor("wdT", [L, FL, D], BF16, kind="ExternalInput")
    embT_ext = nc.dram_tensor("embT", [D, VL], BF16, kind="ExternalInput")
    cos_ext = nc.dram_tensor("cosT", [EL, S], BF16, kind="ExternalInput")
    sin_ext = nc.dram_tensor("sinT", [EL, S], BF16, kind="ExternalInput")
    mask_ext = nc.dram_tensor("maskT", [4, 128, 512], BF16, kind="ExternalInput")
    logits_ext = nc.dram_tensor("logits", [S, VL], BF16, kind="ExternalOutput")

    cc_a_in = nc.dram_tensor("cc_a_in", [NCH, D, SW], BF16)
    cc_a_out = nc.dram_tensor("cc_a_out", [NCH, D, SW], BF16, addr_space="Shared")
    cc_f_in = nc.dram_tensor("cc_f_in", [NCH, D, SW], BF16)
    cc_f_out = nc.dram_tensor("cc_f_out", [NCH, D, SW], BF16, addr_space="Shared")
    RG = [list(range(NC_CORES))]

    SCH = SW // 512              # 512-chunks per half (2)
    TH = SW // 128               # 128-tiles per half (8)

    dbg_tensors = {}

    with tile.TileContext(nc) as tc, ExitStack() as ctx:

        def dbg(name, ap):
            if not DEBUG or name in dbg_tensors:
                return
            ext = nc.dram_tensor("dbg_" + name, list(ap.shape), ap.dtype,
                                 kind="ExternalOutput")
            nc.sync.dma_start(ext[...], ap)
            dbg_tensors[name] = ext
        const_p = ctx.enter_context(tc.tile_pool(name="const", bufs=1))
        persist_p = ctx.enter_context(tc.tile_pool(name="persist", bufs=1))
        work_p = ctx.enter_context(tc.tile_pool(name="work", bufs=2))

        hT = persist_p.tile([128, ND, S], BF16)     # residual stream, d-major
        nc.sync.dma_start(hT[:], hid_ext[:, :].rearrange("(c p) s -> p c s", p=128))
        xT = persist_p.tile([128, ND, S], BF16)     # normed input, d-major

        cos_sb = const_p.tile([EL, S], BF16)
        nc.sync.dma_start(cos_sb[:], cos_ext[:, :])
        sin_sb = const_p.tile([EL, S], BF16)
        nc.sync.dma_start(sin_sb[:], sin_ext[:, :])
        mask_sb = const_p.tile([128, 4, 512], BF16)
        nc.sync.dma_start(mask_sb[:], mask_ext[:, :, :].rearrange("i p b -> p i b"))
        ones_sb = const_p.tile([128, 1], BF16)
        nc.gpsimd.memset(ones_sb[:], 1.0)
        onesr = const_p.tile([1, 128], F32)
        nc.gpsimd.memset(onesr[:], 1.0)
        eps1 = const_p.tile([1, 1], F32)
        nc.gpsimd.memset(eps1[:], EPS)

        VH = VL // 2
        emb_p = ctx.enter_context(tc.tile_pool(name="embp", bufs=1))
        emb_tiles = {}

        def norm_half(g, pool=None):
            """xT[:, :, g-half] = hT / rms(hT) for the s-columns of half g.

            1/rms via Ln+Exp: stays in the natural_log_exp ACT table set
            (shared with attention's Exp) so per-chunk interleaving of norms
            and attention causes no table swaps."""
            gsl = ts(g, SW)
            with ExitStack() as nctx:
                if pool is None:
                    nps = nctx.enter_context(
                        tc.tile_pool(name="nps", bufs=1, space="PSUM"))
                    ssq = nps.tile([1, SW], F32, tag="ssq")
                else:
                    ssq = pool.tile([1, SW], F32, tag="vv", bufs=1, name="ssq")
                sqs = []
                for dc in range(ND):
                    sq = work_p.tile([128, SW], BF16, tag="sq", bufs=3)
                    nc.scalar.activation(sq[:], hT[:, dc, gsl], AF.Square)
                    sqs.append(sq)
                for blk in range(SCH):
                    for dc in range(ND):
                        nc.tensor.matmul(ssq[0:1, ts(blk, 512)], ones_sb[:],
                                         sqs[dc][:, ts(blk, 512)],
                                         start=(dc == 0), stop=(dc == ND - 1))
                rms = work_p.tile([1, SW], F32, tag="rms", bufs=1)
                nc.scalar.activation(rms[:], ssq[:], AF.Sqrt, scale=1.0 / D,
                                     bias=eps1[:])
                inv = work_p.tile([1, SW], F32, tag="inv", bufs=1)
                nc.vector.reciprocal_approx_fast(inv[:], rms[:])
                binv = work_p.tile([128, SW], F32, tag="binv_sb", bufs=2)
                nc.gpsimd.partition_broadcast(binv[:], inv[:], channels=128)
                for dc in range(ND):
                    nc.vector.tensor_tensor(xT[:, dc, gsl], hT[:, dc, gsl],
                                            binv[:], ALU.mult)

        # initial norm (layer-0 attn input; attn_norm_w folded into Wq/Wk/Wv)
        for g in range(NCH):
            norm_half(g)
        dbg("xT0", xT[:])

        with ExitStack() as lctx:
            loop_p = lctx.enter_context(tc.tile_pool(name="loop", bufs=1))
            w_p = lctx.enter_context(tc.tile_pool(name="wts", bufs=1))

            qsb = loop_p.tile([EL, S], BF16)
            ksb = loop_p.tile([EL, S], BF16)
            o_in = loop_p.tile([EL, S], BF16)
            v_store = loop_p.tile([128, NT, HL, 65], BF16)
            nc.gpsimd.memset(v_store[:, :, :, 64:65], 1.0)

            def rope(src_ps, perm_ps, dst, g):
                # src_ps/perm_ps: [128, SW] fp32 psum (raw and 32-block-swapped
                # projections, both computed on PE); dst cols of half g
                for scc in range(SCH):
                    sl = ts(g * SCH + scc, 512)     # S-space slice
                    pl = ts(scc, 512)               # psum slice
                    tq = work_p.tile([128, 512], BF16, tag="ropet")
                    nc.vector.tensor_tensor(tq[:], src_ps[:, pl], cos_sb[:, sl],
                                            ALU.mult)
                    u = work_p.tile([128, 512], BF16, tag="ropeu")
                    nc.vector.tensor_tensor(u[:], perm_ps[:, pl], sin_sb[:, sl],
                                            ALU.mult)
                    nc.vector.tensor_tensor(dst[:, sl], tq[:], u[:], ALU.add)

            def oproj_and_ar(wo_sb, pool, g):
                par = work_p.tile([128, ND, SW], BF16, tag="par", bufs=1)
                for et in range(ND):
                    for scc in range(SCH):
                        sc = g * SCH + scc
                        ppt = pool.tile([128, 512], F32, tag="pps")
                        nc.tensor.matmul(ppt[:], wo_sb[:, ts(et, 128)],
                                         o_in[:, ts(sc, 512)], start=True, stop=True)
                        nc.vector.scalar_tensor_tensor(
                            par[:, et, ts(scc, 512)], hT[:, et, ts(sc, 512)],
                            1.0 / NC_CORES, ppt[:], ALU.mult, ALU.add)
                nc.sync.dma_start(cc_a_in[g].rearrange("(c p) s -> p c s", p=128),
                                  par[:])
                nc.gpsimd.collective_compute(
                    "AllReduce", ALU.add, replica_groups=RG,
                    ins=[cc_a_in[g].opt()], outs=[cc_a_out[g].opt()])

            def qkv_chunk(qkv_ps, g, W, qk_bufs=4, vv_bufs=2):
                qps = qkv_ps.tile([128, SW], F32, tag="qk", bufs=qk_bufs,
                                  name="qps")
                qpps = qkv_ps.tile([128, SW], F32, tag="qk", bufs=qk_bufs,
                                   name="qpps")
                for dc in range(ND):
                    for scc in range(SCH):
                        nc.tensor.matmul(qps[:, ts(scc, 512)], W["wq"][:, dc, :],
                                         xT[:, dc, ts(g * SCH + scc, 512)],
                                         start=(dc == 0), stop=(dc == ND - 1))
                        nc.tensor.matmul(qpps[:, ts(scc, 512)], W["wqp"][:, dc, :],
                                         xT[:, dc, ts(g * SCH + scc, 512)],
                                         start=(dc == 0), stop=(dc == ND - 1))
                rope(qps, qpps, qsb, g)
                kps = qkv_ps.tile([128, SW], F32, tag="qk", bufs=qk_bufs,
                                  name="kps")
                kpps = qkv_ps.tile([128, SW], F32, tag="qk", bufs=qk_bufs,
                                   name="kpps")
                for dc in range(ND):
                    for scc in range(SCH):
                        nc.tensor.matmul(kps[:, ts(scc, 512)], W["wk"][:, dc, :],
                                         xT[:, dc, ts(g * SCH + scc, 512)],
                                         start=(dc == 0), stop=(dc == ND - 1))
                        nc.tensor.matmul(kpps[:, ts(scc, 512)], W["wkp"][:, dc, :],
                                         xT[:, dc, ts(g * SCH + scc, 512)],
                                         start=(dc == 0), stop=(dc == ND - 1))
                rope(kps, kpps, ksb, g)
                vps = qkv_ps.tile([128, TH, HL, 64], F32, tag="vv", bufs=vv_bufs,
                                  name="vps")
                for tt in range(TH):
                    t = g * TH + tt
                    for dc in range(ND):
                        nc.tensor.matmul(vps[:, tt, :, :],
                                         xT[:, dc, ts(t, 128)], W["wv"][:, dc, :],
                                         start=(dc == 0), stop=(dc == ND - 1))
                for tt in range(TH):
                    t = g * TH + tt
                    nc.vector.tensor_copy(v_store[:, t, :, 0:64],
                                          vps[:, tt, :, :])

            def attn_j(j, sc_ps, av_ps, op_ps, wo_sb):
                np_pairs = 2 * j + 2
                for h in range(HL):
                    hb = 64 * h
                    avp = av_ps.tile([65, 512], F32, tag="av", bufs=2,
                                     name="avp")
                    for p in range(np_pairs):
                        kc0, kc1 = 2 * p, 2 * p + 1
                        scp = sc_ps.tile([128, 2, 512], F32, tag="sc",
                                         name="scp")
                        nc.tensor.matmul(scp[:, 0, :],
                                         ksb[hb:hb + 64, ts(kc0, 128)],
                                         qsb[hb:hb + 64, ts(j, 512)],
                                         start=True, stop=True)
                        nc.tensor.matmul(scp[:, 1, :],
                                         ksb[hb:hb + 64, ts(kc1, 128)],
                                         qsb[hb:hb + 64, ts(j, 512)],
                                         start=True, stop=True)
                        psb = work_p.tile([128, 2, 512], BF16, tag="p", bufs=3)
                        nc.scalar.activation(psb[:], scp[:], AF.Exp,
                                             scale=0.125)
                        if p >= 2 * j:      # diagonal pairs: causal mask
                            i0 = 2 * (p - 2 * j)
                            nc.vector.tensor_tensor(
                                psb[:], psb[:],
                                mask_sb[:, i0:i0 + 2, :], ALU.mult)
                        nc.tensor.matmul(avp[:], v_store[:, kc0, h, :],
                                         psb[:, 0, :],
                                         start=(p == 0), stop=False)
                        nc.tensor.matmul(avp[:], v_store[:, kc1, h, :],
                                         psb[:, 1, :],
                                         start=False, stop=(p == np_pairs - 1))
                    srow = work_p.tile([1, 512], F32, tag="srow", bufs=1)
                    nc.vector.tensor_copy(srow[:], avp[64:65, :])
                    srec = work_p.tile([1, 512], F32, tag="srec", bufs=1)
                    nc.vector.reciprocal_approx_fast(srec[:], srow[:])
                    bcsb = work_p.tile([64, 512], F32, tag="bcsb", bufs=1)
                    nc.gpsimd.partition_broadcast(bcsb[:], srec[:], channels=64)
                    nc.vector.tensor_tensor(o_in[hb:hb + 64, ts(j, 512)],
                                            avp[0:64, :], bcsb[:], ALU.mult)
                oproj_and_ar(wo_sb, op_ps, j)

            def load_qkv_weights(l):
                wq_sb = w_p.tile([128, ND, EL], BF16, tag="wq", name="wq_sb")
                nc.sync.dma_start(wq_sb[:], wq_ext[l].rearrange("(c p) e -> p c e", p=128))
                wk_sb = w_p.tile([128, ND, EL], BF16, tag="wk", name="wk_sb")
                nc.sync.dma_start(wk_sb[:], wk_ext[l].rearrange("(c p) e -> p c e", p=128))
                wqp_sb = w_p.tile([128, ND, EL], BF16, tag="wqp", name="wqp_sb")
                nc.sync.dma_start(wqp_sb[:], wqp_ext[l].rearrange("(c p) e -> p c e", p=128))
                wkp_sb = w_p.tile([128, ND, EL], BF16, tag="wkp", name="wkp_sb")
                nc.sync.dma_start(wkp_sb[:], wkp_ext[l].rearrange("(c p) e -> p c e", p=128))
                wv_sb = w_p.tile([128, ND, EL], BF16, tag="wv", name="wv_sb")
                nc.sync.dma_start(wv_sb[:], wv_ext[l].rearrange("(c p) e -> p c e", p=128))
                wo_sb = w_p.tile([EL, D], BF16, tag="wo", name="wo_sb")
                nc.sync.dma_start(wo_sb[:], wo_ext[l])
                return {"wq": wq_sb, "wk": wk_sb, "wqp": wqp_sb,
                        "wkp": wkp_sb, "wv": wv_sb, "wo": wo_sb}

            def load_ffn_weights(l, W):
                wg_sb = w_p.tile([128, ND, FL], BF16, tag="wg", name="wg_sb")
                nc.sync.dma_start(wg_sb[:], wg_ext[l].rearrange("(c p) f -> p c f", p=128))
                wu_sb = w_p.tile([128, ND, FL], BF16, tag="wu", name="wu_sb")
                nc.sync.dma_start(wu_sb[:], wu_ext[l].rearrange("(c p) f -> p c f", p=128))
                wd_sb = w_p.tile([128, NFT, D], BF16, tag="wd", name="wd_sb")
                nc.sync.dma_start(wd_sb[:], wd_ext[l].rearrange("(c p) e -> p c e", p=128))
                W.update({"wg": wg_sb, "wu": wu_sb, "wd": wd_sb})

            W = None
            for l in range(L):
                if l == 1:
                    emb0_sb = emb_p.tile([128, ND, VH], BF16, tag="emb")
                    nc.sync.dma_start(
                        emb0_sb[:],
                        embT_ext[:, 0:VH].rearrange("(c p) v -> p c v", p=128))
                    emb_tiles[0] = emb0_sb
                W = load_qkv_weights(l)
                load_ffn_weights(l, W)

                # ---- per chunk: next-layer norm (from AR_f of l-1) + QKV ----
                with tc.tile_pool(name="qkvps", bufs=2, space="PSUM") as qkv_ps:
                    for g in range(NCH):
                        if l > 0:
                            nc.gpsimd.dma_start(
                                hT[:, :, ts(g, SW)],
                                cc_f_out[g].rearrange("(c p) s -> p c s", p=128))
                            norm_half(g)
                        qkv_chunk(qkv_ps, g, W)

                # ---- attention + o_proj (per q-chunk pipelined with AR) ----
                with tc.tile_pool(name="scps", bufs=2, space="PSUM") as sc_ps, \
                     tc.tile_pool(name="avps", bufs=2, space="PSUM") as av_ps, \
                     tc.tile_pool(name="opps", bufs=2, space="PSUM") as op_ps:
                    for j in range(NSC):
                        attn_j(j, sc_ps, av_ps, op_ps, W["wo"])

                # ---- post-AR_a: norm2 + FFN per half ----
                def ffn_half(g, Wl):
                    gsl = ts(g, SW)
                    gsc = work_p.tile([128, NFT, SW], BF16, tag="gsc", bufs=1)
                    with tc.tile_pool(name="gups", bufs=2, space="PSUM") as gu_ps:
                        for ft in range(NFT):
                            gps = gu_ps.tile([128, SW], F32, tag="gu")
                            for dc in range(ND):
                                for scc in range(SCH):
                                    nc.tensor.matmul(gps[:, ts(scc, 512)],
                                                     Wl["wg"][:, dc, ts(ft, 128)],
                                                     xT[:, dc, ts(g * SCH + scc, 512)],
                                                     start=(dc == 0), stop=(dc == ND - 1))
                            sg = work_p.tile([128, SW], BF16, tag="sg", bufs=2)
                            nc.scalar.activation(sg[:], gps[:], AF.Silu)
                            ups = gu_ps.tile([128, SW], F32, tag="gu")
                            for dc in range(ND):
                                for scc in range(SCH):
                                    nc.tensor.matmul(ups[:, ts(scc, 512)],
                                                     Wl["wu"][:, dc, ts(ft, 128)],
                                                     xT[:, dc, ts(g * SCH + scc, 512)],
                                                     start=(dc == 0), stop=(dc == ND - 1))
                            nc.vector.tensor_tensor(gsc[:, ft, :], ups[:], sg[:],
                                                    ALU.mult)
                    with tc.tile_pool(name="dwps", bufs=2, space="PSUM") as dw_ps:
                        par = work_p.tile([128, ND, SW], BF16, tag="par", bufs=1)
                        for et in range(ND):
                            for scc in range(SCH):
                                dps = dw_ps.tile([128, 512], F32, tag="dw")
                                for fc in range(NFT):
                                    nc.tensor.matmul(dps[:], Wl["wd"][:, fc, ts(et, 128)],
                                                     gsc[:, fc, ts(scc, 512)],
                                                     start=(fc == 0), stop=(fc == NFT - 1))
                                nc.vector.scalar_tensor_tensor(
                                    par[:, et, ts(scc, 512)],
                                    hT[:, et, ts(g * SCH + scc, 512)],
                                    1.0 / NC_CORES, dps[:], ALU.mult, ALU.add)
                        nc.sync.dma_start(
                            cc_f_in[g].rearrange("(c p) s -> p c s", p=128), par[:])
                    nc.gpsimd.collective_compute(
                        "AllReduce", ALU.add, replica_groups=RG,
                        ins=[cc_f_in[g].opt()], outs=[cc_f_out[g].opt()])

                for g in range(NCH):
                    nc.gpsimd.dma_start(
                        hT[:, :, ts(g, SW)],
                        cc_a_out[g].rearrange("(c p) s -> p c s", p=128))
                    norm_half(g)
                    ffn_half(g, W)

        # ---- lm_head (final_norm_w folded into embT); vocab in halves ----
        vchunks = []
        vv = 0
        while vv < VH:
            vchunks.append((vv, min(512, VH - vv)))
            vv += 512
        TPG = NT // NCH

        def lm_t(lps, t, v0, emb_sb):
            lp = lps.tile([128, VH], F32, tag="lm")
            for dc in range(ND):
                for (vv, vn) in vchunks:
                    nc.tensor.matmul(lp[:, vv:vv + vn],
                                     xT[:, dc, ts(t, 128)],
                                     emb_sb[:, dc, vv:vv + vn],
                                     start=(dc == 0), stop=(dc == ND - 1))
            lsb = work_p.tile([128, VH], BF16, tag="lsb", bufs=1)
            nc.scalar.activation(lsb[:, 0:1024], lp[:, 0:1024], AF.Copy)
            nc.vector.tensor_copy(lsb[:, 1024:VH], lp[:, 1024:VH])
            nc.sync.dma_start(logits_ext[ts(t, 128), v0:v0 + VH], lsb[:])

        for g in range(NCH):
            nc.gpsimd.dma_start(
                hT[:, :, ts(g, SW)],
                cc_f_out[g].rearrange("(c p) s -> p c s", p=128))
            norm_half(g)
            with tc.tile_pool(name="lmps", bufs=2, space="PSUM") as lps:
                for t in range(g * TPG, (g + 1) * TPG):
                    lm_t(lps, t, 0, emb_tiles[0])
        emb1_sb = emb_p.tile([128, ND, VH], BF16, tag="emb")
        nc.sync.dma_start(
            emb1_sb[:], embT_ext[:, VH:VL].rearrange("(c p) v -> p c v", p=128))
        with tc.tile_pool(name="lmps", bufs=2, space="PSUM") as lps:
            for t in range(NT):
                lm_t(lps, t, VH, emb1_sb)

    nc.compile()
    return nc


def host_prep(inputs):
    """Full inputs -> per-core in_maps (list of dicts of np arrays)."""
    HD, HL, EL, FL, VL, NT, NSC, ND, NFT = _dims()
    emb = np.ascontiguousarray(np.asarray(inputs["emb"], np.float32))
    ids = np.asarray(inputs["input_ids"]).reshape(-1)
    hid0T = np.ascontiguousarray(emb[ids].T).astype(BF)   # [D, S]

    anw = np.asarray(inputs["attn_norm_w"], np.float32)
    fnw = np.asarray(inputs["ffn_norm_w"], np.float32)
    finw = np.asarray(inputs["final_norm_w"], np.float32)
    Wq = np.asarray(inputs["Wq"], np.float32)
    Wk = np.asarray(inputs["Wk"], np.float32)
    Wv = np.asarray(inputs["Wv"], np.float32)
    Wo = np.asarray(inputs["Wo"], np.float32)
    Wg = np.asarray(inputs["Wg"], np.float32)
    Wu = np.asarray(inputs["Wu"], np.float32)
    Wd = np.asarray(inputs["Wd"], np.float32)

    # rope tables [EL, S]
    inv_freq = 1.0 / (ROPE_BASE ** (np.arange(0, HD, 2, dtype=np.float32) / HD))
    ang = np.arange(S, dtype=np.float32)[:, None] * inv_freq[None, :]   # [S, HD/2]
    ang = np.concatenate([ang, ang], axis=1)                            # [S, HD]
    cosT = np.cos(ang).T.astype(np.float32)                             # [HD, S]
    sinT = np.sin(ang).T.astype(np.float32)
    sinT[:HD // 2] *= -1.0
    cos_full = np.tile(cosT, (HL, 1)).astype(BF)
    sin_full = np.tile(sinT, (HL, 1)).astype(BF)

    # causal masks [4, 128, 512]: multiplicative (1 = keep, 0 = drop)
    a = np.arange(128)[:, None]
    b = np.arange(512)[None, :]
    maskT = np.stack([(a + 128 * i <= b) for i in range(4)]).astype(np.float32)
    maskT = maskT.astype(BF)

    in_maps = []
    for c in range(NC_CORES):
        er = slice(c * EL, (c + 1) * EL)
        fr = slice(c * FL, (c + 1) * FL)
        vr = slice(c * VL, (c + 1) * VL)
        wqT = np.stack([(Wq[l][er, :] * anw[l][None, :]).T for l in range(L)])
        wkT = np.stack([(Wk[l][er, :] * anw[l][None, :]).T for l in range(L)])
        # 32-block-swapped column permutation (rotate-half partner rows)
        perm = np.concatenate([np.arange(32, 64), np.arange(0, 32),
                               np.arange(96, 128), np.arange(64, 96)])
        wqpT = wqT[:, :, perm]
        wkpT = wkT[:, :, perm]
        wvT = np.stack([(Wv[l][er, :] * anw[l][None, :]).T for l in range(L)])
        woT = np.stack([np.ascontiguousarray(Wo[l][:, er].T) for l in range(L)])
        wgT = np.stack([Wg[l][:, fr] * fnw[l][:, None] for l in range(L)])
        wuT = np.stack([Wu[l][:, fr] * fnw[l][:, None] for l in range(L)])
        wdT = np.stack([Wd[l][fr, :] for l in range(L)])
        embT = np.ascontiguousarray((emb[vr, :] * finw[None, :]).T)
        in_maps.append({
            "hid0T": hid0T,
            "wqT": wqT.astype(BF), "wkT": wkT.astype(BF), "wvT": wvT.astype(BF),
            "wqpT": wqpT.astype(BF), "wkpT": wkpT.astype(BF),
            "woT": woT.astype(BF), "wgT": wgT.astype(BF), "wuT": wuT.astype(BF),
            "wdT": wdT.astype(BF), "embT": embT.astype(BF),
            "cosT": cos_full, "sinT": sin_full, "maskT": maskT,
        })
    return in_maps


_RUNNER = None


def make_runner(nc):
    """Wrap a compiled Bacc module into a jitted 8-core callable."""
    import jax
    from jax.sharding import Mesh, PartitionSpec
    from jax.experimental.shard_map import shard_map
    from concourse.bass2jax import (_bass_exec_p, partition_id_tensor,
                                    install_neuronx_cc_hook)
    import jax.numpy as jnp

    install_neuronx_cc_hook()

    partition_name = nc.partition_id_tensor.name if nc.partition_id_tensor else None
    in_names, out_names, out_avals = [], [], []
    for alloc in nc.m.functions[0].allocations:
        if not isinstance(alloc, mybir.MemoryLocationSet):
            continue
        name = alloc.memorylocations[0].name
        if alloc.kind == "ExternalInput":
            if name != partition_name:
                in_names.append(name)
        elif alloc.kind == "ExternalOutput":
            out_names.append(name)
            out_avals.append(jax.core.ShapedArray(
                tuple(alloc.tensor_shape), mybir.dt.np(alloc.dtype)))
    n_params = len(in_names)
    in_names_all = list(in_names) + list(out_names)
    if partition_name is not None:
        in_names_all.append(partition_name)

    def _body(*args):
        operands = list(args)
        if partition_name is not None:
            operands.append(partition_id_tensor())
        outs = _bass_exec_p.bind(
            *operands,
            out_avals=tuple(out_avals),
            in_names=tuple(in_names_all),
            out_names=tuple(out_names),
            lowering_input_output_aliases=(),
            sim_require_finite=True,
            sim_require_nnan=True,
            nc=nc,
        )
        return tuple(outs)

    devices = jax.devices()[:NC_CORES]
    mesh = Mesh(np.asarray(devices), ("core",))
    n_outs = len(out_names)
    in_specs = (PartitionSpec("core"),) * (n_params + n_outs)
    out_specs = (PartitionSpec("core"),) * len(out_names)
    sharded = jax.jit(shard_map(_body, mesh=mesh, in_specs=in_specs,
                                out_specs=out_specs, check_rep=False),
                      keep_unused=True)

    def zero_outs():
        return [np.zeros((NC_CORES * av.shape[0], *av.shape[1:]), av.dtype)
                for av in out_avals]

    def run(in_maps):
        concat_in = [np.concatenate([np.asarray(in_maps[c][nm])
                                     for c in range(NC_CORES)], axis=0)
                     for nm in in_names]
        out_arrs = sharded(*concat_in, *zero_outs())
        import jax as _jax
        _jax.block_until_ready(out_arrs)
        return [
            {nm: np.asarray(out_arrs[i]).reshape(NC_CORES, *out_avals[i].shape)[c]
             for i, nm in enumerate(out_names)}
            for c in range(NC_CORES)
        ]

    run.zero_outs = zero_outs

    run.sharded = sharded
    run.in_names = in_names
    run.out_names = out_names
    run.out_avals = out_avals
    run.mesh = mesh
    run.nc = nc
    return run


def _get_runner():
    global _RUNNER
    if _RUNNER is None:
        _RUNNER = make_runner(build_nc())
    return _RUNNER


def kernel(**inputs) -> np.ndarray:
    HD, HL, EL, FL, VL, NT, NSC, ND, NFT = _dims()
    in_maps = host_prep(inputs)
    run = _get_runner()
    results = run(in_maps)
    logits = np.concatenate([results[c]["logits"].astype(np.float32)
                             for c in range(NC_CORES)], axis=1)
    return logits.reshape(B, S, V)
